# revision 27
# baseline (speedup 1.0000x reference)
"""Trainium2 Bass kernel for nn_MidBlock (ResNet -> Attention -> ResNet).

Data-parallel over batch: 16 images -> 8 cores x 2 images.

Convs use Winograd F(2x2, 3x3): 2.25x less tensor work than direct conv.
Activations live in parity-split padded frames [128, chi, par, 34, 17]
(par = even/odd frame column) so both input-transform passes are
contiguous bf16 DVE ops.  The output transform's first stage (over u) is
folded into PSUM accumulation using host-side sign-fused weights; the
second stage (over v) runs on DVE with ACT evacuating the shared P-tile.
Image 1 runs two weight-chunks behind image 0 so each image's
GN/silu/transform chain hides under the other image's matmuls.

Attention runs in fp8e4 with double-pumped matmuls.  Scores are computed
as hc^T (Wk^T Wq) hc with the weight product folded on the host (biases
are zero), exp'd unnormalized into fp8, and the softmax row-sum
reciprocal is applied after the proj matmul (scaling commutes through
the channel contraction).
"""

import contextlib
import os

import numpy as np
import ml_dtypes

import concourse.bacc as bacc
import concourse.bass as bass
import concourse.tile as tile
from concourse import mybir
from concourse.bass_utils import run_bass_kernel_spmd

F32 = mybir.dt.float32
BF16 = mybir.dt.bfloat16
FP8 = mybir.dt.float8e4
AF = mybir.ActivationFunctionType
OP = mybir.AluOpType
AX = mybir.AxisListType
PM = mybir.MatmulPerfMode

N_CORES = 8
C = 512
B = 16
B_LOC = B // N_CORES
NCHI = 4
EPS = 1e-6
GCNT = 16 * 32 * 32  # elements per group
SA = 128.0  # fp8 scale for the fused scores weight (Wk^T Wq)
SV = 32.0   # fp8 scale for v weights
SH = 0.5    # extra scale-down of stored h~ (fp8e4 max is only 240)
SP = 64.0   # fp8 scale for proj weights (undone via rowsum ones = SP)
EXP_SCALE = 1.0 / (SA * float(np.sqrt(C)))

# consts tile column map (CT [128, 80] fp32)
CB = {"r1c1": 0, "r1c2": 4, "r2c1": 8, "r2c2": 12}
GN_COLS = {"r1g1": (16, 20), "r1g2": (24, 28), "att": (32, 36),
           "r2g1": (40, 44), "r2g2": (48, 52)}
A_COL = 56
PB_COL = 76


def _build(num_devices):
    nc = bacc.Bacc("TRN2", target_bir_lowering=False, debug=False,
                   num_devices=num_devices)
    x_d = nc.dram_tensor("x_fr", [128, B_LOC, NCHI, 2, 34, 17], BF16,
                         kind="ExternalInput").ap()
    wc = {k: nc.dram_tensor(f"w_{k}", [128, 8, 4, 1536], BF16,
                            kind="ExternalInput").ap()
          for k in ("r1c1", "r1c2", "r2c1", "r2c2")}
    wa_d = nc.dram_tensor("wqkvp", [128, 4, NCHI, C], FP8,
                          kind="ExternalInput").ap()
    ct_d = nc.dram_tensor("consts", [128, 80], F32, kind="ExternalInput").ap()
    c8_d = nc.dram_tensor("c8", [128, 128], FP8, kind="ExternalInput").ap()
    atm_d = nc.dram_tensor("atm", [8, 128], F32, kind="ExternalInput").ap()
    out_d = nc.dram_tensor("out", [128, B_LOC, NCHI, 2, 34, 17], BF16,
                           kind="ExternalOutput").ap()

    with tile.TileContext(nc) as tc, contextlib.ExitStack() as ctx:
        pers = ctx.enter_context(tc.tile_pool(name="pers", bufs=1))
        scr = ctx.enter_context(tc.tile_pool(name="scr", bufs=1))
        wpool = ctx.enter_context(tc.tile_pool(name="wpool", bufs=1))
        cpool = ctx.enter_context(tc.tile_pool(name="cpool", bufs=1))
        spool = ctx.enter_context(tc.tile_pool(name="spool", bufs=1))
        apool = ctx.enter_context(tc.tile_pool(name="apool", bufs=1))
        psum = ctx.enter_context(tc.tile_pool(name="psum", bufs=1, space="PSUM"))

        # ---- persistent tiles + input DMAs ----
        XF = [pers.tile([128, NCHI, 2, 34, 17], BF16, tag=f"xf{b}",
                        name=f"xf{b}") for b in range(B_LOC)]
        for b, eng in ((0, nc.sync), (1, nc.gpsimd)):
            for chi in range(NCHI):
                eng.dma_start(out=XF[b][:, chi], in_=x_d[:, b, chi])

        CT = cpool.tile([128, 80], F32, tag="ct", name="ct")
        nc.sync.dma_start(out=CT, in_=ct_d)
        C8 = cpool.tile([128, 128], FP8, tag="c8", name="c8")
        nc.sync.dma_start(out=C8, in_=c8_d)
        ATM = cpool.tile([8, 128], F32, tag="atm", name="atm")
        nc.sync.dma_start(out=ATM, in_=atm_d)
        WA = cpool.tile([128, 4, NCHI, C], FP8, tag="wa", name="wa")
        nc.scalar.dma_start(out=WA, in_=wa_d)

        HB = [scr.tile([128, NCHI, 2, 34, 17], BF16, tag=f"hb{b}",
                       name=f"hb{b}") for b in range(B_LOC)]
        HF = [scr.tile([128, NCHI, 1024], BF16, tag=f"h{b}", name=f"h{b}")
              for b in range(B_LOC)]
        U = [scr.tile([128, 4, 4, NCHI, 16, 16], BF16, tag=f"u{b}",
                      name=f"u{b}") for b in range(B_LOC)]
        SQS = scr.tile([128, 1024], BF16, tag="sqs", name="sqs")  # square sink

        # ---------------- Winograd input transform ----------------
        def tfq(b, frame, utile, v):
            """One v-quarter of the input transform: pass1 col-combo v, then
            the four pass2 row-combos feeding utile[:, :, v]."""
            ev0 = frame[:, :, 0, :, 0:16]
            ev1 = frame[:, :, 0, :, 1:17]
            od0 = frame[:, :, 1, :, 0:16]
            od1 = frame[:, :, 1, :, 1:17]
            p1 = [(ev0, ev1, OP.subtract), (od0, ev1, OP.add),
                  (ev1, od0, OP.subtract), (od0, od1, OP.subtract)]
            p2 = [(0, 2, OP.subtract), (1, 2, OP.add), (2, 1, OP.subtract),
                  (1, 3, OP.subtract)]
            i0, i1, op = p1[v]
            Yv = scr.tile([128, NCHI, 34, 16], BF16, tag="y", name="y",
                          bufs=1)
            nc.vector.tensor_tensor(out=Yv, in0=i0, in1=i1, op=op)

            def rows(a):
                base = a - (a % 2)
                sv = Yv[:, :, base:base + 32, :]
                sv = sv.rearrange("p c (tr two) w -> p c tr two w", two=2)
                return sv[:, :, :, a % 2, :]

            for u, (a0, a1, op2) in enumerate(p2):
                nc.vector.tensor_tensor(out=utile[:, u, v], in0=rows(a0),
                                        in1=rows(a1), op=op2)

        def tf2(b, frame, utile):
            for v in range(4):
                tfq(b, frame, utile, v)

        # ---------------- conv (Winograd matmuls + stage2) ----------------
        def wchunk(key, co, i, vh, eng=None):
            """half-chunk: weights for (co, i) and v in {2*vh, 2*vh+1}"""
            eng = eng or nc.sync
            wt = wpool.tile([128, 2, 3, NCHI, 128], BF16, tag="wt", name="wt",
                            bufs=6)
            coi = co * 2 + i
            for v2 in range(2):
                fv = wt[:, v2].rearrange("p t c n -> p (t c n)")
                v = 2 * vh + v2
                eng.dma_start(out=fv[:, 0:768], in_=wc[key][:, coi, v, 0:768])
                eng.dma_start(out=fv[:, 768:1536],
                              in_=wc[key][:, coi, v, 768:1536])
            return wt

        def conv_mm_half(P, wt, b, vh, i):
            for v2 in range(2):
                v = 2 * vh + v2
                for t in range(3):
                    u = i + t
                    for chi in range(NCHI):
                        nc.tensor.matmul(
                            P[:, v], wt[:, v2, t, chi, :],
                            U[b][:, u, v, chi],
                            start=(t == 0 and chi == 0),
                            stop=(t == 2 and chi == NCHI - 1))

        def stage2(P, b, co, i, bias_col):
            e1b = spool.tile([128, 256], BF16, tag="e1", name="e1", bufs=2)
            e2 = spool.tile([128, 256], BF16, tag="e2", name="e2", bufs=2)
            t0 = spool.tile([128, 256], BF16, tag="t0", name="t0", bufs=2)
            t1 = spool.tile([128, 256], BF16, tag="t1", name="t1", bufs=2)
            nc.scalar.activation(out=e1b, in_=P[:, 1], func=AF.Identity,
                                 bias=CT[:, bias_col:bias_col + 1], scale=1.0)
            nc.scalar.activation(out=e2, in_=P[:, 2], func=AF.Identity)
            o0 = HF[b][:, co, i * 512: i * 512 + 256]
            o1 = HF[b][:, co, i * 512 + 256: i * 512 + 512]
            nc.vector.tensor_tensor(out=t0, in0=P[:, 0], in1=e1b, op=OP.add)
            nc.vector.tensor_tensor(out=o0, in0=t0, in1=e2, op=OP.add)
            nc.gpsimd.tensor_tensor(out=t1, in0=e1b, in1=e2, op=OP.subtract)
            nc.vector.tensor_tensor(out=o1, in0=t1, in1=P[:, 3], op=OP.subtract)

        def conv_block(key, stats_hook, post0=None, post1=None,
                       pre_img1=None, iter_hooks=None):
            """Winograd conv for both images; img1 skewed 4 weight
            half-chunks (2 full chunks) behind img0 so img0's post-chain
            overlaps img1's matmuls and post1 overlaps the next block's
            img0 work."""
            bias0 = CB[key]
            halves = [(co, i, vh) for co in range(4) for i in range(2)
                      for vh in range(2)]
            wts = {}
            for c in range(4):
                wts[c] = wchunk(key, *halves[c])
            Pcur = {}
            for k in range(20):
                if k < 16:
                    co, i, vh = halves[k]
                    if vh == 0:
                        Pcur[0] = psum.tile([128, 4, 256], F32, tag="cv",
                                            name="cv", bufs=2)
                    conv_mm_half(Pcur[0], wts[k], 0, vh, i)
                    if vh == 1:
                        stage2(Pcur[0], 0, co, i, bias0 + co)
                        if i == 1:
                            stats_hook(0, co)
                if k == 1 and pre_img1 is not None:
                    pre_img1()
                if k >= 4:
                    co, i, vh = halves[k - 4]
                    if vh == 0:
                        Pcur[1] = psum.tile([128, 4, 256], F32, tag="cv",
                                            name="cv", bufs=2)
                    conv_mm_half(Pcur[1], wts[k - 4], 1, vh, i)
                    if vh == 1:
                        stage2(Pcur[1], 1, co, i, bias0 + co)
                        if i == 1:
                            stats_hook(1, co)
                if k == 15 and post0 is not None:
                    post0()
                if k >= 2 and k + 2 < 16:
                    wts[k + 2] = wchunk(key, *halves[k + 2])
                if iter_hooks is not None and k in iter_hooks:
                    iter_hooks[k]()
            if post1 is not None:
                post1()

        # ---------------- group norm ----------------
        def gn_stats_h(b, co, ST):
            nc.vector.reduce_sum(out=ST[:, co:co + 1], in_=HF[b][:, co],
                                 axis=AX.X)
            nc.scalar.activation(out=SQS, in_=HF[b][:, co], func=AF.Square,
                                 accum_out=ST[:, 4 + co:5 + co])

        def gn_stats_xf(b, ST):
            for chi in range(NCHI):
                for par in range(2):
                    col = chi * 2 + par
                    k0 = 1 - par
                    v = XF[b][:, chi, par, 1:33, k0:k0 + 16]
                    nc.vector.reduce_sum(out=ST[:, col:col + 1], in_=v,
                                         axis=AX.XY)
                    nc.scalar.activation(
                        out=SQS[:, 0:512].rearrange("p (r w) -> p r w", w=16),
                        in_=v, func=AF.Square,
                        accum_out=ST[:, 8 + col:9 + col])

        def gn_chain(ST, gkey, ncols=8):
            """ST [128, 8|16] -> per-channel scale/shift SC/TC [128, 4]."""
            gcol, bcol = GN_COLS[gkey]
            G = psum.tile([128, 16], F32, tag="gp", name="gp", bufs=1)
            nc.tensor.matmul(G[:8, :ncols], CT[:, A_COL:A_COL + 8],
                             ST[:, :ncols], start=True, stop=True)
            SG = spool.tile([8, 8], F32, tag="sg", name="sg", bufs=4)
            T8 = spool.tile([8, 4], F32, tag="t8", name="t8", bufs=4)
            if ncols == 16:
                GS = spool.tile([8, 16], F32, tag="gs", name="gs", bufs=2)
                nc.vector.tensor_copy(out=GS, in_=G[:8, 0:16])
                gv = GS.rearrange("p (c two) -> p c two", two=2)
                nc.vector.tensor_tensor(out=SG, in0=gv[:, :, 0],
                                        in1=gv[:, :, 1], op=OP.add)
                nc.vector.tensor_scalar_mul(out=SG, in0=SG, scalar1=1.0 / GCNT)
            else:
                nc.vector.tensor_scalar_mul(out=SG, in0=G[:8, :8],
                                            scalar1=1.0 / GCNT)
            nc.vector.tensor_mul(out=T8, in0=SG[:, 0:4], in1=SG[:, 0:4])
            nc.vector.tensor_tensor(out=SG[:, 4:8], in0=SG[:, 4:8], in1=T8,
                                    op=OP.subtract)
            # rstd via DVE fast-rsqrt + 1 Newton step
            nc.vector.tensor_scalar_add(out=SG[:, 4:8], in0=SG[:, 4:8],
                                        scalar1=EPS)
            Y8 = spool.tile([8, 4], F32, tag="y8", name="y8", bufs=4)
            vi = SG[:, 4:8].bitcast(mybir.dt.uint32)
            yi = Y8.bitcast(mybir.dt.uint32)
            nc.vector.tensor_scalar(out=yi, in0=vi, scalar1=1, scalar2=None,
                                    op0=OP.logical_shift_right)
            nc.vector.tensor_scalar(out=yi, in0=yi, scalar1=-1,
                                    scalar2=0x5F3759DF, op0=OP.mult, op1=OP.add)
            nc.vector.tensor_mul(out=T8, in0=Y8, in1=Y8)
            nc.vector.tensor_mul(out=T8, in0=T8, in1=SG[:, 4:8])
            nc.vector.tensor_scalar(out=T8, in0=T8, scalar1=-0.5,
                                    scalar2=1.5, op0=OP.mult, op1=OP.add)
            nc.vector.tensor_mul(out=Y8, in0=Y8, in1=T8)
            nc.vector.tensor_copy(out=SG[:, 4:8], in_=Y8)
            MBp = psum.tile([128, 16], F32, tag="gp", name="gp", bufs=1)
            nc.tensor.matmul(MBp[:, :8], ATM, SG, start=True, stop=True)
            MB = spool.tile([128, 8], F32, tag="mb", name="mb", bufs=4)
            nc.vector.tensor_copy(out=MB, in_=MBp[:, :8])
            SC = spool.tile([128, 4], F32, tag="sc", name="sc", bufs=4)
            TC = spool.tile([128, 4], F32, tag="tc", name="tc", bufs=4)
            nc.vector.tensor_mul(out=SC, in0=MB[:, 4:8], in1=CT[:, gcol:gcol + 4])
            nc.vector.tensor_mul(out=TC, in0=MB[:, 0:4], in1=SC)
            nc.vector.tensor_tensor(out=TC, in0=CT[:, bcol:bcol + 4], in1=TC,
                                    op=OP.subtract)
            return SC, TC

        def frame_memset_borders(frame):
            nc.gpsimd.memset(frame[:, :, :, 0, :], 0.0)
            nc.gpsimd.memset(frame[:, :, :, 33, :], 0.0)
            nc.gpsimd.memset(frame[:, :, 0, :, 0:1], 0.0)
            nc.gpsimd.memset(frame[:, :, 1, :, 16:17], 0.0)

        def silu_to_frame(b, SC, TC, frame):
            """silu(SC*h+TC) -> frame interior (next conv's input)."""
            for co in range(4):
                hv = HF[b][:, co].rearrange("p (i j n) -> p i j n", i=2, j=2)
                for j in range(2):
                    par = 1 - j  # j=0 -> odd cols (par1), j=1 -> even (par0)
                    k0 = 0 if j == 0 else 1
                    ov = frame[:, co, par, 1:33, k0:k0 + 16]
                    ov = ov.rearrange("p (tr two) w -> p tr two w", two=2)
                    iv = hv[:, :, j].rearrange("p i (tr w) -> p i tr w", w=16)
                    iv = iv.rearrange("p i tr w -> p tr i w")
                    nc.scalar.activation(
                        out=ov, in_=iv, func=AF.Silu,
                        bias=TC[:, co:co + 1], scale=SC[:, co:co + 1])

        def silu_flat(b, SC, TC, dst):
            for co in range(4):
                nc.scalar.activation(
                    out=dst[:, co], in_=HF[b][:, co], func=AF.Silu,
                    bias=TC[:, co:co + 1], scale=SC[:, co:co + 1])

        def skip_add(b, sf):
            """XF[b] += sf (flat, (i,j,tr,tc) pixel order)."""
            for chi in range(NCHI):
                for j in range(2):
                    par = 1 - j
                    k0 = 0 if j == 0 else 1
                    ov = XF[b][:, chi, par, 1:33, k0:k0 + 16]
                    ov = ov.rearrange("p (tr two) w -> p tr two w", two=2)
                    hv = sf[:, chi].rearrange("p (i j n) -> p i j n", i=2, j=2)
                    iv = hv[:, :, j].rearrange("p i (tr w) -> p i tr w", w=16)
                    iv = iv.rearrange("p i tr w -> p tr i w")
                    nc.vector.tensor_tensor(out=ov, in0=iv, in1=ov, op=OP.add)

        # ---------------- attention ----------------
        AG = apool.tile([128, NCHI, 1024], FP8, tag="ag", name="ag")
        AV = apool.tile([128, 8, 512], FP8, tag="av", name="av")
        AE = apool.tile([128, 8, 1024], FP8, tag="ae", name="ae")
        AH = apool.tile([128, NCHI, 1024], FP8, tag="ah", name="ah")
        RB = apool.tile([128, 2, 512], BF16, tag="rb", name="rb")
        hcs = [apool.tile([128, NCHI, 1024], FP8, tag=f"hc{b}",
                          name=f"hc{b}") for b in range(B_LOC)]

        def att_hc(b, SC, TC):
            """hc = fp8(GN-linear(XF[b])), pixel order n = par*512 + r*16 + c"""
            for chi in range(NCHI):
                for par in range(2):
                    k0 = 1 - par
                    iv = XF[b][:, chi, par, 1:33, k0:k0 + 16]
                    ovv = hcs[b][:, chi, par * 512:par * 512 + 512]
                    ovv = ovv.rearrange("p (r w) -> p r w", w=16)
                    nc.vector.tensor_scalar(
                        out=ovv, in0=iv, scalar1=SC[:, chi:chi + 1],
                        scalar2=TC[:, chi:chi + 1], op0=OP.mult, op1=OP.add)

        def att_g1(b):
            """G1 = (SA * Wk^T Wq) @ hc; scores are then hc^T G1."""
            for co in range(4):
                for h2 in range(2):
                    P = psum.tile([128, 512], F32, tag="ap", name="ap", bufs=2)
                    for pr in range(2):
                        nc.tensor.matmul(
                            P, WA[:, 0, 2 * pr:2 * pr + 2,
                                  co * 128:(co + 1) * 128],
                            hcs[b][:, 2 * pr:2 * pr + 2,
                                   h2 * 512:(h2 + 1) * 512],
                            start=pr == 0, stop=pr == 1,
                            perf_mode=PM.DoubleRow)
                    nc.scalar.activation(
                        out=AG[:, co, h2 * 512:(h2 + 1) * 512], in_=P,
                        func=AF.Identity)

        def att_v(b):
            for mb in range(8):
                P = psum.tile([128, 512], F32, tag="ap", name="ap", bufs=2)
                for pr in range(2):
                    nc.tensor.matmul(
                        P, hcs[b][:, 2 * pr:2 * pr + 2, mb * 128:(mb + 1) * 128],
                        WA[:, 2, 2 * pr:2 * pr + 2, :],
                        start=pr == 0, stop=pr == 1, perf_mode=PM.DoubleRow)
                nc.scalar.activation(out=AV[:, mb], in_=P, func=AF.Identity)

        def att_scores(b):
            for mb in range(8):
                for h2 in range(2):
                    P = psum.tile([128, 512], F32, tag="ap", name="ap", bufs=2)
                    for pr in range(2):
                        nc.tensor.matmul(
                            P, hcs[b][:, 2 * pr:2 * pr + 2,
                                      mb * 128:(mb + 1) * 128],
                            AG[:, 2 * pr:2 * pr + 2, h2 * 512:(h2 + 1) * 512],
                            start=pr == 0, stop=pr == 1,
                            perf_mode=PM.DoubleRow)
                    nc.scalar.activation(
                        out=AE[:, mb, h2 * 512:(h2 + 1) * 512], in_=P,
                        func=AF.Exp, scale=EXP_SCALE)

        def att_sums(b):
            # ones-matmul with a full [128,128] SP matrix: every psum
            # partition holds SP * rowsum, so no partition broadcast needed
            for h2 in range(2):
                RS = psum.tile([128, 512], F32, tag="rs", name="rs", bufs=1)
                for mb in range(8):
                    nc.tensor.matmul(RS, C8,
                                     AE[:, mb, h2 * 512:(h2 + 1) * 512],
                                     start=mb == 0, stop=mb == 7)
                with nc.allow_low_precision(reason="softmax 1/rowsum bf16"):
                    nc.vector.reciprocal(RB[:, h2], RS)

        def att_av(b):
            for cb in range(4):
                for h2 in range(2):
                    P = psum.tile([128, 512], F32, tag="ap", name="ap", bufs=2)
                    for pr in range(4):
                        nc.tensor.matmul(
                            P, AV[:, 2 * pr:2 * pr + 2, cb * 128:(cb + 1) * 128],
                            AE[:, 2 * pr:2 * pr + 2, h2 * 512:(h2 + 1) * 512],
                            start=pr == 0, stop=pr == 3,
                            perf_mode=PM.DoubleRow)
                    nc.vector.tensor_scalar(
                        out=AH[:, cb, h2 * 512:(h2 + 1) * 512], in0=P,
                        scalar1=SH / SV, scalar2=0.0, op0=OP.mult, op1=OP.add)

        def att_proj(b):
            for co in range(4):
                for h2 in range(2):
                    P = psum.tile([128, 512], F32, tag="ap", name="ap", bufs=2)
                    for pr in range(2):
                        nc.tensor.matmul(
                            P, WA[:, 3, 2 * pr:2 * pr + 2,
                                  co * 128:(co + 1) * 128],
                            AH[:, 2 * pr:2 * pr + 2, h2 * 512:(h2 + 1) * 512],
                            start=pr == 0, stop=pr == 1,
                            perf_mode=PM.DoubleRow)
                    tmul = spool.tile([128, 512], BF16, tag="tm", name="tm",
                                      bufs=1)
                    nc.vector.tensor_tensor(out=tmul, in0=P, in1=RB[:, h2],
                                            op=OP.mult)
                    par = h2
                    k0 = 1 - par
                    ov = XF[b][:, co, par, 1:33, k0:k0 + 16]
                    iv = tmul.rearrange("p (r w) -> p r w", w=16)
                    nc.vector.scalar_tensor_tensor(
                        out=ov, in0=iv,
                        scalar=CT[:, PB_COL + co:PB_COL + co + 1],
                        in1=ov, op0=OP.add, op1=OP.add)

        # =================== schedule ===================
        ST_r = [spool.tile([128, 8], F32, tag=f"st{b}", name=f"st{b}", bufs=4)
                for b in range(B_LOC)]
        ST_a = [spool.tile([128, 16], F32, tag=f"sta{b}", name=f"sta{b}",
                           bufs=2) for b in range(B_LOC)]

        def att_pre(b):
            gn_stats_xf(b, ST_a[b])
            SC, TC = gn_chain(ST_a[b], "att", ncols=16)
            att_hc(b, SC, TC)

        def att_av_half(b, cbs):
            for cb in cbs:
                for h2 in range(2):
                    P = psum.tile([128, 512], F32, tag="ap", name="ap", bufs=2)
                    for pr in range(4):
                        nc.tensor.matmul(
                            P, AV[:, 2 * pr:2 * pr + 2, cb * 128:(cb + 1) * 128],
                            AE[:, 2 * pr:2 * pr + 2, h2 * 512:(h2 + 1) * 512],
                            start=pr == 0, stop=pr == 3,
                            perf_mode=PM.DoubleRow)
                    nc.vector.tensor_scalar(
                        out=AH[:, cb, h2 * 512:(h2 + 1) * 512], in0=P,
                        scalar1=SH / SV, scalar2=0.0, op0=OP.mult, op1=OP.add)

        def att_main():
            # ordered so img1 writes to shared tiles follow img0's readers;
            # img0's r2-transform quarters are spread through img1's phases
            att_g1(0)
            att_v(0)
            att_scores(0)
            att_g1(1)
            att_sums(0)
            att_av(0)
            att_v(1)
            att_scores(1)
            att_proj(0)
            tfq(0, XF[0], U[0], 0)
            att_sums(1)
            tfq(0, XF[0], U[0], 1)
            att_av_half(1, (0, 1))
            tfq(0, XF[0], U[0], 2)
            att_av_half(1, (2, 3))
            tfq(0, XF[0], U[0], 3)
            att_proj(1)

        def resnet(blk, final0, final1, c1_hooks=None):
            c1, g1k, c2, g2k = blk + "c1", blk + "g1", blk + "c2", blk + "g2"

            def stats(b, co):
                gn_stats_h(b, co, ST_r[b])

            def post_c1(b):
                SC, TC = gn_chain(ST_r[b], g1k)
                frame_memset_borders(HB[b])
                silu_to_frame(b, SC, TC, HB[b])

            def post_c2(b, final):
                SC, TC = gn_chain(ST_r[b], g2k)
                silu_flat(b, SC, TC, HF[b])
                skip_add(b, HF[b])
                if final is not None:
                    final()

            # img0's c2-transform quarters run in c1's tail iterations;
            # img1's in c2's head iterations
            hooks1 = dict(c1_hooks or {})
            for v in range(4):
                hooks1[16 + v] = (lambda v=v: tfq(0, HB[0], U[0], v))
            conv_block(c1, stats, post0=lambda: post_c1(0),
                       post1=lambda: post_c1(1), iter_hooks=hooks1)
            hooks2 = {v: (lambda v=v: tfq(1, HB[1], U[1], v))
                      for v in range(4)}
            conv_block(c2, stats,
                       post0=lambda: post_c2(0, final0),
                       post1=lambda: post_c2(1, final1), iter_hooks=hooks2)

        def dma_out(b):
            eng = nc.sync if b == 0 else nc.gpsimd
            for chi in range(NCHI):
                eng.dma_start(out=out_d[:, b, chi], in_=XF[b][:, chi])

        def r1_final1():
            att_pre(1)
            att_main()

        DBG = int(os.environ.get("KDBG", "0"))
        r1c1_hooks = {v: (lambda v=v: tfq(1, XF[1], U[1], v))
                      for v in range(4)}
        tf2(0, XF[0], U[0])
        if DBG == 1:  # resnet1 only
            resnet("r1", lambda: dma_out(0), lambda: dma_out(1))
        elif DBG == 2:  # resnet1 + attention, with intermediate dumps
            dbg_g = nc.dram_tensor("dbg_g", [128, NCHI, 1024], FP8,
                                   kind="ExternalOutput").ap()
            dbg_e = nc.dram_tensor("dbg_e", [128, 8, 1024], FP8,
                                   kind="ExternalOutput").ap()
            dbg_r = nc.dram_tensor("dbg_r", [128, 2, 512], BF16,
                                   kind="ExternalOutput").ap()
            dbg_h = nc.dram_tensor("dbg_h", [128, NCHI, 1024], FP8,
                                   kind="ExternalOutput").ap()
            dbg_hc = nc.dram_tensor("dbg_hc", [128, NCHI, 1024], FP8,
                                    kind="ExternalOutput").ap()

            def fin1():
                att_pre(1)
                att_g1(0)
                nc.sync.dma_start(out=dbg_g, in_=AG)
                nc.sync.dma_start(out=dbg_hc, in_=hcs[0])
                att_v(0)
                att_scores(0)
                nc.sync.dma_start(out=dbg_e, in_=AE)
                att_sums(0)
                nc.sync.dma_start(out=dbg_r, in_=RB)
                att_av(0)
                nc.sync.dma_start(out=dbg_h, in_=AH)
                att_scores(1) if False else None
                att_proj(0)
                dma_out(0)
                dma_out(1)
            resnet("r1", lambda: att_pre(0), fin1)
        elif DBG == 4:  # attention img1 only, serial
            def fin1d():
                att_pre(1)
                att_g1(1)
                att_v(1)
                att_scores(1)
                att_sums(1)
                att_av(1)
                att_proj(1)
                dma_out(0)
                dma_out(1)
            resnet("r1", None, fin1d)
        else:
            resnet("r1", lambda: att_pre(0), r1_final1, c1_hooks=r1c1_hooks)
            resnet("r2", lambda: dma_out(0), lambda: dma_out(1),
                   c1_hooks={v + 1: (lambda v=v: tfq(1, XF[1], U[1], v))
                             for v in range(4)})

    nc.compile()
    return nc


# ====================== host side ======================

def _prep_inputs(inputs):
    f32 = np.float32
    bf = ml_dtypes.bfloat16
    f8 = ml_dtypes.float8_e4m3

    for k in ("a_qb", "a_kb", "a_vb"):
        assert np.abs(np.asarray(inputs[k], f32)).max() == 0.0, \
            f"nonzero {k} not supported by the fused fp8 attention path"

    x = np.asarray(inputs["x"], f32)
    xp = np.zeros((N_CORES, B_LOC, NCHI, 128, 34, 34), f32)
    xp[:, :, :, :, 1:33, 1:33] = x.reshape(N_CORES, B_LOC, NCHI, 128, 32, 32)
    fr = np.stack([xp[..., 0::2], xp[..., 1::2]], axis=4)
    # fr: [cores, b, chi, p, par, 34, 17] -> [cores, p, b, chi, par, 34, 17]
    x_fr = np.ascontiguousarray(fr.transpose(0, 3, 1, 2, 4, 5, 6)).astype(bf)

    G = np.array([[1, 0, 0], [.5, .5, .5], [.5, -.5, .5], [0, 0, 1]], np.float64)

    def wino(w):
        w = np.asarray(w, f32).astype(np.float64)  # [co, ci, 3, 3]
        wt = np.einsum('ua,vb,oiab->uvio', G, G, w)  # [4,4,ci,co]
        wf = np.zeros((2, 3, 4, C, C), np.float64)  # [i, t, v, ci, co]
        for i in range(2):
            for t in range(3):
                u = i + t
                sgn = 1.0 if (i == 0 or t == 0) else -1.0
                wf[i, t] = sgn * wt[u]
        wf = wf.reshape(2, 3, 4, NCHI, 128, 4, 128)  # i,t,v,chi,p,cob,cs
        wf = wf.transpose(4, 5, 0, 2, 1, 3, 6)  # p, cob, i, v, t, chi, cs
        wf = wf.reshape(128, 8, 4, 1536)
        return np.ascontiguousarray(wf).astype(bf)

    def onew(w, scale):
        # [out, in] -> lhsT [p(ci), chi, co] with scale
        w = np.asarray(w, f32).T * scale  # [ci, co]
        return w.reshape(NCHI, 128, C).transpose(1, 0, 2)

    wq = np.asarray(inputs["a_qw"], f32)
    wk = np.asarray(inputs["a_kw"], f32)
    amat = wk.T @ wq  # A[j, i]: scores^T = (A @ hc) paired with hc
    wg = onew(amat, SA)  # lhsT[i, j] = SA * A[j, i] = SA * A.T
    wv = onew(inputs["a_vw"], SV)
    wp = onew(inputs["a_pw"], SP)
    wqkvp = np.ascontiguousarray(
        np.stack([wg, np.zeros_like(wg), wv, wp], axis=1)).astype(f8)

    def col(v):
        return np.asarray(v, f32).reshape(NCHI, 128).T

    ct = np.zeros((128, 80), np.float32)
    ct[:, 0:4] = col(inputs["r1_c1b"])
    ct[:, 4:8] = col(inputs["r1_c2b"])
    ct[:, 8:12] = col(inputs["r2_c1b"])
    ct[:, 12:16] = col(inputs["r2_c2b"])
    for (g, bta), (gc, bc) in zip(
            [("r1_g1", "r1_b1"), ("r1_g2", "r1_b2"), ("a_g", "a_b"),
             ("r2_g1", "r2_b1"), ("r2_g2", "r2_b2")],
            [GN_COLS[k] for k in ("r1g1", "r1g2", "att", "r2g1", "r2g2")]):
        ct[:, gc:gc + 4] = col(inputs[g])
        ct[:, bc:bc + 4] = col(inputs[bta])
    p_idx = np.arange(128)
    ct[:, A_COL:A_COL + 8] = (p_idx[:, None] // 16 == np.arange(8)[None, :])
    ct[:, PB_COL:PB_COL + 4] = col(inputs["a_pb"])

    c8 = np.full((128, 128), SP * SH, np.float32).astype(f8)
    atm = np.ascontiguousarray(
        (np.arange(8)[:, None] == p_idx[None, :] // 16).astype(np.float32))

    shared = {
        "w_r1c1": wino(inputs["r1_c1w"]), "w_r1c2": wino(inputs["r1_c2w"]),
        "w_r2c1": wino(inputs["r2_c1w"]), "w_r2c2": wino(inputs["r2_c2w"]),
        "wqkvp": wqkvp, "consts": ct, "c8": c8, "atm": atm,
    }
    in_maps = [dict(shared, x_fr=np.ascontiguousarray(x_fr[i]))
               for i in range(N_CORES)]
    return in_maps


_NC_CACHE = {}


def _get_nc(num_devices=N_CORES):
    if num_devices not in _NC_CACHE:
        _NC_CACHE[num_devices] = _build(num_devices)
    return _NC_CACHE[num_devices]


def _gather(results):
    outs = [np.asarray(r["out"]) for r in results]
    y = np.stack(outs, axis=0).astype(np.float32)
    # y: [cores, 128, b, chi, par, 34, 17]
    y = y[:, :, :, :, :, 1:33, :]  # valid rows
    out = np.empty((N_CORES, 128, B_LOC, NCHI, 32, 32), np.float32)
    out[..., 1::2] = y[:, :, :, :, 0, :, 1:17]  # even frame cols -> img 1,3..31
    out[..., 0::2] = y[:, :, :, :, 1, :, 0:16]  # odd frame cols -> img 0,2..30
    out = out.transpose(0, 2, 3, 1, 4, 5).reshape(B, C, 32, 32)
    return np.ascontiguousarray(out)


def kernel(**inputs):
    nc = _get_nc()
    in_maps = _prep_inputs(inputs)
    res = run_bass_kernel_spmd(nc, in_maps, core_ids=list(range(N_CORES)))
    return _gather(res.results)
